# revision 15
# baseline (speedup 1.0000x reference)
"""LowBitEncoder Trainium2 kernel.

y = LayerNorm((x @ tern(W).T + bias) * scale) -> tanh(y/qs) -> round-to-1/127 grid.
Data-parallel: batch dim (8) sharded across 8 NeuronCores; weight replicated.

Wall-clock of kernel() is dominated by host<->device transfer, so the wire
format is minimized:
  x    shipped as float16   [T, DIN]      (16 MiB/core)
  W    ternarized on host, packed 2-bit into W^T layout [DIN, DOUT/4] u8
       (4 MiB/core; field k of byte j = tern_w[o = k*1024 + j, d] + 1)
  out  shipped back as int8 k = round(tanh*127); host computes k/127.0f
       (8 MiB/core, bit-exact reconstruction)

Per-core pipeline:
  prep:  DVE-unpack wp -> W^T f16 [DIN, DOUT] in DRAM scratch
  main:  per token block: x^T via XBAR dma-transpose (f16);
         stream W^T slabs via plain DMA; f16 matmuls accumulate
         y[tile, 4096] in 8 PSUM banks; DVE evac (+row sums),
         ACT square (+row sumsq), LN normalize, ACT tanh(scale=1/qs),
         round via magic-number trick, cast int8, DMA out.
"""
import sys
import time
import numpy as np
from contextlib import ExitStack

import concourse.bass as bass
from concourse import bacc
import concourse.tile as tile
import concourse.mybir as mybir
from concourse.bass_utils import run_bass_kernel_spmd

B, S, DIN, DOUT = 8, 2048, 4096, 4096
P = 128
T = S                 # tokens per core (batch-sharded)
NCORES = 8
THRESH = 0.1
LN_EPS = 1e-5
MAGIC = 12582912.0    # 1.5 * 2**23: round-half-even for |v| < 2**22
f32, f16 = mybir.dt.float32, mybir.dt.float16
u8, i8 = mybir.dt.uint8, mybir.dt.int8
Alu = mybir.AluOpType
Act = mybir.ActivationFunctionType

_CACHE = {}


def _build(trivial_params: bool):
    """Build the Bass program. trivial_params: bias==0, scale==1, gamma==1, beta==0."""
    T_B = 512 if trivial_params else 256       # tokens per block
    NBLK = T // T_B
    NTT = T_B // P                             # t-tiles per block (4 or 2)
    KT = DIN // P                              # 32 k-tiles
    NOS = 8 // NTT                             # PSUM banks per t-tile
    OPW = NOS * 512                            # o-columns per phase
    NOP = DOUT // OPW                          # o-phases
    QW = DOUT // 4                             # packed bytes per row
    NSH = DIN // NCORES                        # W^T rows unpacked per core

    nc = bacc.Bacc("TRN2", target_bir_lowering=False, debug=False,
                   num_devices=NCORES)
    x_d = nc.dram_tensor("x", [T, DIN], f16, kind="ExternalInput")
    wp_d = nc.dram_tensor("wp", [NSH, QW], u8, kind="ExternalInput")
    qs_d = nc.dram_tensor("qs", [1], f32, kind="ExternalInput")
    if not trivial_params:
        bias_d = nc.dram_tensor("bias", [DOUT], f32, kind="ExternalInput")
        scale_d = nc.dram_tensor("scale", [DOUT], f32, kind="ExternalInput")
        gam_d = nc.dram_tensor("gam", [DOUT], f32, kind="ExternalInput")
        bet_d = nc.dram_tensor("bet", [DOUT], f32, kind="ExternalInput")
    out_d = nc.dram_tensor("out", [T, DOUT], i8, kind="ExternalOutput")
    wt_part = nc.dram_tensor("wt_part", [NSH, DOUT], f16)  # this core's shard
    wt_h = nc.dram_tensor("wt_h", [DIN, DOUT], f16,
                          addr_space="Shared")        # all-gathered W^T

    with tile.TileContext(nc) as tc:
        with ExitStack() as ctx:
            consts = ctx.enter_context(tc.tile_pool(name="consts", bufs=1))
            wprep = ctx.enter_context(tc.tile_pool(name="wprep", bufs=2))
            xt_pool = ctx.enter_context(tc.tile_pool(name="xt", bufs=2))
            wst = ctx.enter_context(tc.tile_pool(name="wst", bufs=3))
            ypool = ctx.enter_context(tc.tile_pool(name="y", bufs=NTT))
            stat = ctx.enter_context(tc.tile_pool(name="stat", bufs=2 * NTT + 2))
            sq_pool = ctx.enter_context(tc.tile_pool(name="sq", bufs=2))
            opool = ctx.enter_context(tc.tile_pool(name="o", bufs=2))
            pp = ctx.enter_context(tc.tile_pool(name="ps", bufs=8, space="PSUM"))

            # ---- quant scale: [128,1] 1/qs ----
            tqs = consts.tile([P, 1], f32, tag="tqs")
            nc.sync.dma_start(tqs[:], qs_d.ap().partition_broadcast(P))
            tinv = consts.tile([P, 1], f32, tag="tinv")
            nc.vector.reciprocal(tinv[:], tqs[:])
            zero_t = consts.tile([P, 1], f32, tag="zero_t")
            nc.vector.memset(zero_t[:], 0.0)
            eps_t = consts.tile([P, 1], f32, tag="eps_t")
            nc.vector.memset(eps_t[:], LN_EPS)

            # ---- replicated per-channel params (general path only) ----
            if not trivial_params:
                s_rep = consts.tile([P, DOUT], f32, tag="s_rep")
                nc.sync.dma_start(s_rep[:], scale_d.ap().partition_broadcast(P))
                b_rep = consts.tile([P, DOUT], f32, tag="b_rep")
                nc.sync.dma_start(b_rep[:], bias_d.ap().partition_broadcast(P))
                bs_rep = consts.tile([P, DOUT], f32, tag="bs_rep")
                nc.vector.tensor_tensor(bs_rep[:], b_rep[:], s_rep[:], Alu.mult)
                g_rep = consts.tile([P, DOUT], f32, tag="g_rep")
                nc.sync.dma_start(g_rep[:], gam_d.ap().partition_broadcast(P))
                be_rep = consts.tile([P, DOUT], f32, tag="be_rep")
                nc.sync.dma_start(be_rep[:], bet_d.ap().partition_broadcast(P))

            # ---- W prep: unpack this core's 2-bit shard -> f16 W^T rows,
            # then AllGather the full [DIN, DOUT] W^T across the 8 cores ----
            # byte (d, j) field k = tern_w[k*QW + j, d] + 1  (values 0/1/2)
            for rb in range(NSH // P):
                wpt = wprep.tile([P, QW], u8, tag="wpt", name=f"wpt_{rb}")
                nc.sync.dma_start(wpt[:], wp_d.ap()[rb * P:(rb + 1) * P, :])
                wsl = wprep.tile([P, DOUT], f16, tag="wsl", name=f"wsl_{rb}")
                for k in range(4):
                    tmp = wprep.tile([P, QW], u8, tag="wtmp",
                                     name=f"wtmp_{rb}_{k}")
                    nc.vector.tensor_scalar(
                        tmp[:], wpt[:], 2 * k, 3,
                        Alu.logical_shift_right, Alu.bitwise_and)
                    nc.vector.tensor_scalar(
                        wsl[:, k * QW:(k + 1) * QW], tmp[:], 1.0, None,
                        Alu.subtract)
                nc.sync.dma_start(wt_part.ap()[rb * P:(rb + 1) * P, :], wsl[:])
            nc.gpsimd.collective_compute(
                "AllGather", Alu.bypass,
                replica_groups=[list(range(NCORES))],
                ins=[wt_part.ap()], outs=[wt_h.ap()])

            # ---- main loop over token blocks ----
            for blk in range(NBLK):
                t0 = blk * T_B
                # x^T for this block: [128 d, KT, T_B] f16 via XBAR transpose
                xt = xt_pool.tile([P, KT, T_B], f16, tag="xt",
                                  name=f"xt_{blk}")
                for k in range(KT):
                    nc.sync.dma_start_transpose(
                        xt[:, k, :],
                        x_d.ap()[t0:t0 + T_B, k * P:(k + 1) * P])

                ycur = [None] * NTT
                scur = [None] * NTT
                qcur = [None] * NTT
                for op in range(NOP):
                    o0 = op * OPW
                    banks = []
                    for tt in range(NTT):
                        for os_ in range(NOS):
                            bank_t = pp.tile([P, 512], f32, tag="bank",
                                             name=f"bank_{blk}_{op}_{tt}_{os_}")
                            banks.append(bank_t)
                    # stream W^T slabs and accumulate
                    for k in range(KT):
                        wslab = wst.tile([P, OPW], f16, tag="ws",
                                         name=f"ws_{blk}_{op}_{k}")
                        nc.sync.dma_start(
                            wslab[:],
                            wt_h.ap()[k * P:(k + 1) * P, o0:o0 + OPW])
                        for tt in range(NTT):
                            for os_ in range(NOS):
                                nc.tensor.matmul(
                                    banks[tt * NOS + os_][:],
                                    xt[:, k, tt * P:(tt + 1) * P],
                                    wslab[:, os_ * 512:(os_ + 1) * 512],
                                    start=(k == 0), stop=(k == KT - 1))
                    # evacuate + stats
                    for tt in range(NTT):
                        if op == 0:
                            ycur[tt] = ypool.tile([P, DOUT], f32, tag="y",
                                                  name=f"y_{blk}_{tt}")
                            scur[tt] = stat.tile([P, 2 * NOP], f32, tag="sums",
                                                 name=f"sums_{blk}_{tt}")
                            qcur[tt] = stat.tile([P, 2 * NOP], f32, tag="sumsq",
                                                 name=f"sumsq_{blk}_{tt}")
                        y = ycur[tt]; sums = scur[tt]; sumsq = qcur[tt]
                        for os_ in range(NOS):
                            col = op * NOS + os_
                            zsl = y[:, o0 + os_ * 512: o0 + (os_ + 1) * 512]
                            bankap = banks[tt * NOS + os_][:]
                            if trivial_params:
                                nc.vector.tensor_scalar(
                                    zsl, bankap, 1.0, 0.0, Alu.mult, Alu.add,
                                    accum_out=sums[:, col:col + 1])
                            else:
                                zt = sq_pool.tile([P, 512], f32, tag="zt",
                                                  name=f"zt_{blk}_{op}_{tt}_{os_}")
                                nc.vector.tensor_tensor(
                                    zt[:], bankap,
                                    s_rep[:, o0 + os_ * 512: o0 + (os_ + 1) * 512],
                                    Alu.mult)
                                nc.vector.tensor_tensor_reduce(
                                    out=zsl, in0=zt[:],
                                    in1=bs_rep[:, o0 + os_ * 512: o0 + (os_ + 1) * 512],
                                    scale=1.0, scalar=0.0,
                                    op0=Alu.add, op1=Alu.add,
                                    accum_out=sums[:, col:col + 1])
                            sq = sq_pool.tile([P, 512], f32, tag="sq",
                                              name=f"sq_{blk}_{op}_{tt}_{os_}")
                            nc.scalar.activation(
                                sq[:], zsl, Act.Square, bias=zero_t[:, 0:1],
                                accum_out=sumsq[:, col:col + 1])

                # ---- per-t-tile epilogue ----
                for tt in range(NTT):
                    y = ycur[tt]; sums = scur[tt]; sumsq = qcur[tt]
                    mu = stat.tile([P, 1], f32, tag="mu", name=f"mu_{blk}_{tt}")
                    nc.vector.tensor_reduce(
                        out=mu[:], in_=sums[:], op=Alu.add,
                        axis=mybir.AxisListType.X)
                    nc.vector.tensor_scalar(mu[:], mu[:], 1.0 / DOUT, None, Alu.mult)
                    e2 = stat.tile([P, 1], f32, tag="e2", name=f"e2_{blk}_{tt}")
                    nc.vector.tensor_reduce(
                        out=e2[:], in_=sumsq[:], op=Alu.add,
                        axis=mybir.AxisListType.X)
                    musq = stat.tile([P, 1], f32, tag="musq", name=f"musq_{blk}_{tt}")
                    nc.vector.tensor_tensor(musq[:], mu[:], mu[:], Alu.mult)
                    var = stat.tile([P, 1], f32, tag="var", name=f"var_{blk}_{tt}")
                    nc.vector.tensor_scalar(
                        var[:], e2[:], 1.0 / DOUT, None, Alu.mult)
                    nc.vector.tensor_tensor(var[:], var[:], musq[:], Alu.subtract)
                    sd = stat.tile([P, 1], f32, tag="sd", name=f"sd_{blk}_{tt}")
                    nc.scalar.activation(sd[:], var[:], Act.Sqrt, bias=eps_t[:, 0:1])
                    inv = stat.tile([P, 1], f32, tag="inv", name=f"inv_{blk}_{tt}")
                    nc.vector.reciprocal(inv[:], sd[:])
                    # normalize in place: (z - mu) * inv
                    nc.vector.tensor_scalar(
                        y[:], y[:], mu[:, 0:1], inv[:, 0:1],
                        Alu.subtract, Alu.mult)
                    if not trivial_params:
                        nc.vector.tensor_tensor(y[:], y[:], g_rep[:], Alu.mult)
                        nc.vector.tensor_tensor(y[:], y[:], be_rep[:], Alu.add)
                    # tanh(y / qs)
                    nc.scalar.activation(y[:], y[:], Act.Tanh,
                                         bias=zero_t[:, 0:1], scale=tinv[:, 0:1])
                    # k = round(tanh*127) via magic, emitted as int8
                    nc.vector.tensor_scalar(
                        y[:], y[:], 127.0, MAGIC, Alu.mult, Alu.add)
                    oi8 = opool.tile([P, DOUT], i8, tag="oi8",
                                     name=f"oi8_{blk}_{tt}")
                    nc.vector.tensor_scalar(
                        oi8[:], y[:], MAGIC, None, Alu.subtract)
                    nc.sync.dma_start(
                        out_d.ap()[blk * T_B + tt * P: blk * T_B + (tt + 1) * P, :],
                        oi8[:])

    nc.compile()
    return nc


def _pack_ternary(weight: np.ndarray) -> np.ndarray:
    """Ternarize weight [DOUT, DIN] and 2-bit-pack W^T -> [DIN, DOUT//4] u8.

    Field k of byte (d, j) holds tern_w[k*(DOUT//4) + j, d] + 1 (0/1/2).
    Matches reference: tern = where(|w| < THRESH, 0, sign(w)).
    """
    w = np.asarray(weight, dtype=np.float32)
    enc = np.ones(w.shape, dtype=np.uint8)
    enc += w >= THRESH
    enc -= w <= -THRESH
    q = DOUT // 4
    wp2 = (enc[0 * q:1 * q]
           | (enc[1 * q:2 * q] << 2)
           | (enc[2 * q:3 * q] << 4)
           | (enc[3 * q:4 * q] << 6))            # [q, DIN]
    return np.ascontiguousarray(wp2.T)           # [DIN, q]


def kernel(x, weight, bias, scale, ln_gamma, ln_beta, quant_scale):
    times = {}
    t0 = time.perf_counter()
    trivial = (
        not np.any(bias) and not np.any(ln_beta)
        and np.all(scale == 1.0) and np.all(ln_gamma == 1.0)
    )
    if trivial not in _CACHE:
        _CACHE[trivial] = _build(trivial)
    nc = _CACHE[trivial]
    times["build"] = time.perf_counter() - t0

    t0 = time.perf_counter()
    wp = _pack_ternary(weight)
    times["packw"] = time.perf_counter() - t0

    t0 = time.perf_counter()
    xh = np.asarray(x).astype(np.float16).reshape(NCORES, T, DIN)
    times["xf16"] = time.perf_counter() - t0

    qs = np.asarray(quant_scale, dtype=np.float32)
    nsh = DIN // NCORES
    in_maps = []
    for c in range(NCORES):
        im = {"x": xh[c], "wp": wp[c * nsh:(c + 1) * nsh], "qs": qs}
        if not trivial:
            im["bias"] = np.asarray(bias, dtype=np.float32)
            im["scale"] = np.asarray(scale, dtype=np.float32)
            im["gam"] = np.asarray(ln_gamma, dtype=np.float32)
            im["bet"] = np.asarray(ln_beta, dtype=np.float32)
        in_maps.append(im)

    t0 = time.perf_counter()
    res = run_bass_kernel_spmd(nc, in_maps, list(range(NCORES)))
    times["run"] = time.perf_counter() - t0

    t0 = time.perf_counter()
    out = np.empty((NCORES, T, DOUT), np.float32)
    for c in range(NCORES):
        np.divide(res.results[c]["out"], np.float32(127.0), out=out[c])
    out = out.reshape(B, S, DOUT)
    times["outcvt"] = time.perf_counter() - t0
    print("kernel timings: "
          + " ".join(f"{k}={v:.2f}s" for k, v in times.items()),
          file=sys.stderr)
    return out


# revision 17
# speedup vs baseline: 1.0788x; 1.0788x over previous
"""LowBitEncoder Trainium2 kernel.

y = LayerNorm((x @ tern(W).T + bias) * scale) -> tanh(y/qs) -> round-to-1/127 grid.
Data-parallel: batch dim (8) sharded across 8 NeuronCores.

Wall-clock of kernel() is dominated by host<->device transfer, so the wire
format is minimized:
  x    shipped as float16   [T, DIN]         (16 MiB/core)
  W    ternarized on host, packed 2-bit into W^T layout, sharded by rows
       across the cores (0.5 MiB/core) and AllGathered on-fabric after
       on-device unpack to f16
  out  shipped back as int8 k = round(tanh*127); host computes k/127.0f
       (8 MiB/core, bit-exact reconstruction)

Per-core pipeline:
  prep:  DVE-unpack this core's wp shard -> W^T f16 rows in DRAM,
         AllGather to the full [DIN, DOUT] W^T
  main:  per 512-token block: x^T via XBAR dma-transpose (f16);
         stream W^T slabs via plain DMA; f16 matmuls accumulate
         y[tile, 4096] in 8 PSUM banks; DVE evac (+row sums),
         ACT square (+row sumsq), LN normalize, ACT tanh(scale=1/qs),
         round via magic-number trick, cast int8, DMA out.

The affine-params path (bias/scale/gamma/beta not identity) is served by an
exact numpy fallback — the device program is specialized to the common
identity-params case.
"""
import sys
import time
import numpy as np
from contextlib import ExitStack

import concourse.bass as bass
from concourse import bacc
import concourse.tile as tile
import concourse.mybir as mybir
from concourse.bass_utils import run_bass_kernel_spmd

B, S, DIN, DOUT = 8, 2048, 4096, 4096
P = 128
T = S                 # tokens per core (batch-sharded)
NCORES = 8
THRESH = 0.1
LN_EPS = 1e-5
MAGIC = 12582912.0    # 1.5 * 2**23: round-half-even for |v| < 2**22
f32, f16 = mybir.dt.float32, mybir.dt.float16
u8, i8 = mybir.dt.uint8, mybir.dt.int8
Alu = mybir.AluOpType
Act = mybir.ActivationFunctionType

_CACHE = {}


def _build():
    """Bass program for the identity-params case (bias=0, scale=1, g=1, b=0)."""
    T_B = 512                                  # tokens per block
    NBLK = T // T_B
    NTT = T_B // P                             # 4 t-tiles per block
    KT = DIN // P                              # 32 k-tiles
    NOS = 8 // NTT                             # 2 PSUM banks per t-tile
    OPW = NOS * 512                            # 1024 o-columns per phase
    NOP = DOUT // OPW                          # 4 o-phases
    QW = DOUT // 4                             # packed bytes per row
    NSH = DIN // NCORES                        # W^T rows unpacked per core

    nc = bacc.Bacc("TRN2", target_bir_lowering=False, debug=False,
                   num_devices=NCORES)
    x_d = nc.dram_tensor("x", [T, DIN], f16, kind="ExternalInput")
    wp_d = nc.dram_tensor("wp", [NSH, QW], u8, kind="ExternalInput")
    qs_d = nc.dram_tensor("qs", [1], f32, kind="ExternalInput")
    out_d = nc.dram_tensor("out", [T, DOUT], i8, kind="ExternalOutput")
    wt_part = nc.dram_tensor("wt_part", [NSH, DOUT], f16)  # this core's shard
    wt_h = nc.dram_tensor("wt_h", [DIN, DOUT], f16,
                          addr_space="Shared")        # all-gathered W^T

    with tile.TileContext(nc) as tc:
        with ExitStack() as ctx:
            consts = ctx.enter_context(tc.tile_pool(name="consts", bufs=1))
            wprep = ctx.enter_context(tc.tile_pool(name="wprep", bufs=2))
            xt_pool = ctx.enter_context(tc.tile_pool(name="xt", bufs=2))
            wst = ctx.enter_context(tc.tile_pool(name="wst", bufs=3))
            ypool = ctx.enter_context(tc.tile_pool(name="y", bufs=NTT))
            stat = ctx.enter_context(tc.tile_pool(name="stat", bufs=2 * NTT + 2))
            sq_pool = ctx.enter_context(tc.tile_pool(name="sq", bufs=2))
            opool = ctx.enter_context(tc.tile_pool(name="o", bufs=2))
            pp = ctx.enter_context(tc.tile_pool(name="ps", bufs=8, space="PSUM"))

            # ---- quant scale: [128,1] 1/qs ----
            tqs = consts.tile([P, 1], f32, tag="tqs")
            nc.sync.dma_start(tqs[:], qs_d.ap().partition_broadcast(P))
            tinv = consts.tile([P, 1], f32, tag="tinv")
            nc.vector.reciprocal(tinv[:], tqs[:])
            zero_t = consts.tile([P, 1], f32, tag="zero_t")
            nc.vector.memset(zero_t[:], 0.0)
            eps_t = consts.tile([P, 1], f32, tag="eps_t")
            nc.vector.memset(eps_t[:], LN_EPS)

            # ---- W prep: unpack this core's 2-bit shard -> f16 W^T rows,
            # then AllGather the full [DIN, DOUT] W^T across the 8 cores ----
            # byte (d, j) field k = tern_w[k*QW + j, d] + 1  (values 0/1/2)
            for rb in range(NSH // P):
                wpt = wprep.tile([P, QW], u8, tag="wpt", name=f"wpt_{rb}")
                nc.sync.dma_start(wpt[:], wp_d.ap()[rb * P:(rb + 1) * P, :])
                wsl = wprep.tile([P, DOUT], f16, tag="wsl", name=f"wsl_{rb}")
                for k in range(4):
                    tmp = wprep.tile([P, QW], u8, tag="wtmp",
                                     name=f"wtmp_{rb}_{k}")
                    nc.vector.tensor_scalar(
                        tmp[:], wpt[:], 2 * k, 3,
                        Alu.logical_shift_right, Alu.bitwise_and)
                    nc.vector.tensor_scalar(
                        wsl[:, k * QW:(k + 1) * QW], tmp[:], 1.0, None,
                        Alu.subtract)
                nc.sync.dma_start(wt_part.ap()[rb * P:(rb + 1) * P, :], wsl[:])
            nc.gpsimd.collective_compute(
                "AllGather", Alu.bypass,
                replica_groups=[list(range(NCORES))],
                ins=[wt_part.ap()], outs=[wt_h.ap()])

            # ---- main loop over token blocks ----
            for blk in range(NBLK):
                t0 = blk * T_B
                # x^T for this block: [128 d, KT, T_B] f16 via XBAR transpose
                xt = xt_pool.tile([P, KT, T_B], f16, tag="xt",
                                  name=f"xt_{blk}")
                for k in range(KT):
                    nc.sync.dma_start_transpose(
                        xt[:, k, :],
                        x_d.ap()[t0:t0 + T_B, k * P:(k + 1) * P])

                ycur = [None] * NTT
                scur = [None] * NTT
                qcur = [None] * NTT
                for op in range(NOP):
                    o0 = op * OPW
                    banks = []
                    for tt in range(NTT):
                        for os_ in range(NOS):
                            bank_t = pp.tile([P, 512], f32, tag="bank",
                                             name=f"bank_{blk}_{op}_{tt}_{os_}")
                            banks.append(bank_t)
                    # stream W^T slabs and accumulate
                    for k in range(KT):
                        wslab = wst.tile([P, OPW], f16, tag="ws",
                                         name=f"ws_{blk}_{op}_{k}")
                        nc.sync.dma_start(
                            wslab[:],
                            wt_h.ap()[k * P:(k + 1) * P, o0:o0 + OPW])
                        for tt in range(NTT):
                            for os_ in range(NOS):
                                nc.tensor.matmul(
                                    banks[tt * NOS + os_][:],
                                    xt[:, k, tt * P:(tt + 1) * P],
                                    wslab[:, os_ * 512:(os_ + 1) * 512],
                                    start=(k == 0), stop=(k == KT - 1))
                    # evacuate + stats
                    for tt in range(NTT):
                        if op == 0:
                            ycur[tt] = ypool.tile([P, DOUT], f32, tag="y",
                                                  name=f"y_{blk}_{tt}")
                            scur[tt] = stat.tile([P, NOP * NOS], f32,
                                                 tag="sums",
                                                 name=f"sums_{blk}_{tt}")
                            qcur[tt] = stat.tile([P, NOP * NOS], f32,
                                                 tag="sumsq",
                                                 name=f"sumsq_{blk}_{tt}")
                        y = ycur[tt]; sums = scur[tt]; sumsq = qcur[tt]
                        for os_ in range(NOS):
                            col = op * NOS + os_
                            zsl = y[:, o0 + os_ * 512: o0 + (os_ + 1) * 512]
                            bankap = banks[tt * NOS + os_][:]
                            nc.vector.tensor_scalar(
                                zsl, bankap, 1.0, 0.0, Alu.mult, Alu.add,
                                accum_out=sums[:, col:col + 1])
                            sq = sq_pool.tile([P, 512], f32, tag="sq",
                                              name=f"sq_{blk}_{op}_{tt}_{os_}")
                            nc.scalar.activation(
                                sq[:], zsl, Act.Square, bias=zero_t[:, 0:1],
                                accum_out=sumsq[:, col:col + 1])

                # ---- per-t-tile epilogue ----
                for tt in range(NTT):
                    y = ycur[tt]; sums = scur[tt]; sumsq = qcur[tt]
                    mu = stat.tile([P, 1], f32, tag="mu", name=f"mu_{blk}_{tt}")
                    nc.vector.tensor_reduce(
                        out=mu[:], in_=sums[:], op=Alu.add,
                        axis=mybir.AxisListType.X)
                    nc.vector.tensor_scalar(mu[:], mu[:], 1.0 / DOUT, None, Alu.mult)
                    e2 = stat.tile([P, 1], f32, tag="e2", name=f"e2_{blk}_{tt}")
                    nc.vector.tensor_reduce(
                        out=e2[:], in_=sumsq[:], op=Alu.add,
                        axis=mybir.AxisListType.X)
                    musq = stat.tile([P, 1], f32, tag="musq", name=f"musq_{blk}_{tt}")
                    nc.vector.tensor_tensor(musq[:], mu[:], mu[:], Alu.mult)
                    var = stat.tile([P, 1], f32, tag="var", name=f"var_{blk}_{tt}")
                    nc.vector.tensor_scalar(
                        var[:], e2[:], 1.0 / DOUT, None, Alu.mult)
                    nc.vector.tensor_tensor(var[:], var[:], musq[:], Alu.subtract)
                    sd = stat.tile([P, 1], f32, tag="sd", name=f"sd_{blk}_{tt}")
                    nc.scalar.activation(sd[:], var[:], Act.Sqrt, bias=eps_t[:, 0:1])
                    inv = stat.tile([P, 1], f32, tag="inv", name=f"inv_{blk}_{tt}")
                    nc.vector.reciprocal(inv[:], sd[:])
                    # normalize in place: (z - mu) * inv
                    nc.vector.tensor_scalar(
                        y[:], y[:], mu[:, 0:1], inv[:, 0:1],
                        Alu.subtract, Alu.mult)
                    # tanh(y / qs)
                    nc.scalar.activation(y[:], y[:], Act.Tanh,
                                         bias=zero_t[:, 0:1], scale=tinv[:, 0:1])
                    # k = round(tanh*127) via magic, emitted as int8
                    nc.vector.tensor_scalar(
                        y[:], y[:], 127.0, MAGIC, Alu.mult, Alu.add)
                    oi8 = opool.tile([P, DOUT], i8, tag="oi8",
                                     name=f"oi8_{blk}_{tt}")
                    nc.vector.tensor_scalar(
                        oi8[:], y[:], MAGIC, None, Alu.subtract)
                    nc.sync.dma_start(
                        out_d.ap()[blk * T_B + tt * P: blk * T_B + (tt + 1) * P, :],
                        oi8[:])

    nc.compile()
    return nc


def _pack_ternary(weight: np.ndarray) -> np.ndarray:
    """Ternarize weight [DOUT, DIN] and 2-bit-pack W^T -> [DIN, DOUT//4] u8.

    Field k of byte (d, j) holds tern_w[k*(DOUT//4) + j, d] + 1 (0/1/2).
    Matches reference: tern = where(|w| < THRESH, 0, sign(w)).
    """
    w = np.asarray(weight, dtype=np.float32)
    enc = np.ones(w.shape, dtype=np.uint8)
    enc += w >= THRESH
    enc -= w <= -THRESH
    q = DOUT // 4
    wp2 = (enc[0 * q:1 * q]
           | (enc[1 * q:2 * q] << 2)
           | (enc[2 * q:3 * q] << 4)
           | (enc[3 * q:4 * q] << 6))            # [q, DIN]
    return np.ascontiguousarray(wp2.T)           # [DIN, q]


def _host_general(x, weight, bias, scale, ln_gamma, ln_beta, quant_scale):
    """Exact host fallback for the affine-params case."""
    x2 = np.asarray(x, dtype=np.float32).reshape(-1, DIN)
    w = np.asarray(weight, dtype=np.float32)
    tern = np.where(np.abs(w) < THRESH, 0.0, np.sign(w)).astype(np.float32)
    y = x2 @ tern.T
    y += np.asarray(bias, dtype=np.float32)
    y *= np.asarray(scale, dtype=np.float32)
    mu = y.mean(-1, keepdims=True)
    var = y.var(-1, keepdims=True)
    y = (y - mu) / np.sqrt(var + LN_EPS)
    y *= np.asarray(ln_gamma, dtype=np.float32)
    y += np.asarray(ln_beta, dtype=np.float32)
    y = np.tanh(y / np.asarray(quant_scale, dtype=np.float32))
    y = np.round(y * 127.0, out=y)
    y /= np.float32(127.0)
    return y.reshape(B, S, DOUT).astype(np.float32)


def kernel(x, weight, bias, scale, ln_gamma, ln_beta, quant_scale):
    trivial = (
        not np.any(bias) and not np.any(ln_beta)
        and np.all(scale == 1.0) and np.all(ln_gamma == 1.0)
    )
    if not trivial:
        return _host_general(x, weight, bias, scale, ln_gamma, ln_beta,
                             quant_scale)

    times = {}
    t0 = time.perf_counter()
    if "prog" not in _CACHE:
        _CACHE["prog"] = _build()
    nc = _CACHE["prog"]
    times["build"] = time.perf_counter() - t0

    t0 = time.perf_counter()
    wp = _pack_ternary(weight)
    times["packw"] = time.perf_counter() - t0

    t0 = time.perf_counter()
    xh = np.asarray(x).astype(np.float16).reshape(NCORES, T, DIN)
    times["xf16"] = time.perf_counter() - t0

    qs = np.asarray(quant_scale, dtype=np.float32)
    nsh = DIN // NCORES
    in_maps = [
        {"x": xh[c], "wp": wp[c * nsh:(c + 1) * nsh], "qs": qs}
        for c in range(NCORES)
    ]

    t0 = time.perf_counter()
    res = run_bass_kernel_spmd(nc, in_maps, list(range(NCORES)))
    times["run"] = time.perf_counter() - t0

    t0 = time.perf_counter()
    out = np.empty((NCORES, T, DOUT), np.float32)
    for c in range(NCORES):
        np.divide(res.results[c]["out"], np.float32(127.0), out=out[c])
    out = out.reshape(B, S, DOUT)
    times["outcvt"] = time.perf_counter() - t0
    print("kernel timings: "
          + " ".join(f"{k}={v:.2f}s" for k, v in times.items()),
          file=sys.stderr)
    return out


# revision 21
# speedup vs baseline: 1.1764x; 1.0905x over previous
"""LowBitEncoder Trainium2 kernel.

y = LayerNorm((x @ tern(W).T + bias) * scale) -> tanh(y/qs) -> round-to-1/127 grid.
Data-parallel: batch dim (8) sharded across 8 NeuronCores.

Wall-clock of kernel() is dominated by host<->device transfer, so the wire
format is minimized:
  x    shipped as float16   [T, DIN]         (16 MiB/core)
  W    ternarized on host, packed 2-bit into W^T layout, sharded by rows
       across the cores (0.5 MiB/core) and AllGathered on-fabric after
       on-device unpack to f16
  out  shipped back as int8 k = round(tanh*127); host computes k/127.0f
       (8 MiB/core, bit-exact reconstruction)

Per-core pipeline:
  prep:  DVE-unpack this core's wp shard -> W^T f16 rows in DRAM,
         AllGather to the full [DIN, DOUT] W^T
  main:  per 512-token block: x^T via XBAR dma-transpose (f16);
         stream W^T slabs via plain DMA; f16 matmuls accumulate
         y[tile, 4096] in 8 PSUM banks; DVE evac (+row sums),
         ACT square (+row sumsq), LN normalize, ACT tanh(scale=1/qs),
         round via magic-number trick, cast int8, DMA out.

The affine-params path (bias/scale/gamma/beta not identity) is served by an
exact numpy fallback — the device program is specialized to the common
identity-params case.
"""
import sys
import time
import numpy as np
from concurrent.futures import ThreadPoolExecutor
from contextlib import ExitStack

import concourse.bass as bass
from concourse import bacc
import concourse.tile as tile
import concourse.mybir as mybir
from concourse.bass_utils import run_bass_kernel_spmd

B, S, DIN, DOUT = 8, 2048, 4096, 4096
P = 128
T = S                 # tokens per core (batch-sharded)
NCORES = 8
THRESH = 0.1
LN_EPS = 1e-5
MAGIC = 12582912.0    # 1.5 * 2**23: round-half-even for |v| < 2**22
f32, f16 = mybir.dt.float32, mybir.dt.float16
u8, i8 = mybir.dt.uint8, mybir.dt.int8
Alu = mybir.AluOpType
Act = mybir.ActivationFunctionType

_CACHE = {}


NCHUNK = 2            # token-chunks pipelined per call (overlap wire + host)
T_C = T // NCHUNK     # tokens per core per chunk


def _build():
    """Bass program for the identity-params case (bias=0, scale=1, g=1, b=0).

    Covers T_C tokens per core; kernel() runs it NCHUNK times on worker
    threads so chunk k+1's upload overlaps chunk k's download/host work.
    """
    T_B = 512                                  # tokens per block
    NBLK = T_C // T_B
    NTT = T_B // P                             # 4 t-tiles per block
    KT = DIN // P                              # 32 k-tiles
    NOS = 8 // NTT                             # 2 PSUM banks per t-tile
    OPW = NOS * 512                            # 1024 o-columns per phase
    NOP = DOUT // OPW                          # 4 o-phases
    QW = DOUT // 4                             # packed bytes per row
    NSH = DIN // NCORES                        # W^T rows unpacked per core

    nc = bacc.Bacc("TRN2", target_bir_lowering=False, debug=False,
                   num_devices=NCORES)
    x_d = nc.dram_tensor("x", [T_C, DIN], f16, kind="ExternalInput")
    wp_d = nc.dram_tensor("wp", [NSH, QW], u8, kind="ExternalInput")
    qs_d = nc.dram_tensor("qs", [1], f32, kind="ExternalInput")
    out_d = nc.dram_tensor("out", [T_C, DOUT], i8, kind="ExternalOutput")
    wt_part = nc.dram_tensor("wt_part", [NSH, DOUT], f16)  # this core's shard
    wt_h = nc.dram_tensor("wt_h", [DIN, DOUT], f16,
                          addr_space="Shared")        # all-gathered W^T

    with tile.TileContext(nc) as tc:
        with ExitStack() as ctx:
            consts = ctx.enter_context(tc.tile_pool(name="consts", bufs=1))
            wprep = ctx.enter_context(tc.tile_pool(name="wprep", bufs=2))
            xt_pool = ctx.enter_context(tc.tile_pool(name="xt", bufs=2))
            wst = ctx.enter_context(tc.tile_pool(name="wst", bufs=3))
            ypool = ctx.enter_context(tc.tile_pool(name="y", bufs=NTT))
            stat = ctx.enter_context(tc.tile_pool(name="stat", bufs=2 * NTT + 2))
            sq_pool = ctx.enter_context(tc.tile_pool(name="sq", bufs=2))
            opool = ctx.enter_context(tc.tile_pool(name="o", bufs=2))
            pp = ctx.enter_context(tc.tile_pool(name="ps", bufs=8, space="PSUM"))

            # ---- quant scale: [128,1] 1/qs ----
            tqs = consts.tile([P, 1], f32, tag="tqs")
            nc.sync.dma_start(tqs[:], qs_d.ap().partition_broadcast(P))
            tinv = consts.tile([P, 1], f32, tag="tinv")
            nc.vector.reciprocal(tinv[:], tqs[:])
            zero_t = consts.tile([P, 1], f32, tag="zero_t")
            nc.vector.memset(zero_t[:], 0.0)
            eps_t = consts.tile([P, 1], f32, tag="eps_t")
            nc.vector.memset(eps_t[:], LN_EPS)

            # ---- W prep: unpack this core's 2-bit shard -> f16 W^T rows,
            # then AllGather the full [DIN, DOUT] W^T across the 8 cores ----
            # byte (d, j) field k = tern_w[k*QW + j, d] + 1  (values 0/1/2)
            for rb in range(NSH // P):
                wpt = wprep.tile([P, QW], u8, tag="wpt", name=f"wpt_{rb}")
                nc.sync.dma_start(wpt[:], wp_d.ap()[rb * P:(rb + 1) * P, :])
                wsl = wprep.tile([P, DOUT], f16, tag="wsl", name=f"wsl_{rb}")
                for k in range(4):
                    tmp = wprep.tile([P, QW], u8, tag="wtmp",
                                     name=f"wtmp_{rb}_{k}")
                    nc.vector.tensor_scalar(
                        tmp[:], wpt[:], 2 * k, 3,
                        Alu.logical_shift_right, Alu.bitwise_and)
                    nc.vector.tensor_scalar(
                        wsl[:, k * QW:(k + 1) * QW], tmp[:], 1.0, None,
                        Alu.subtract)
                nc.sync.dma_start(wt_part.ap()[rb * P:(rb + 1) * P, :], wsl[:])
            nc.gpsimd.collective_compute(
                "AllGather", Alu.bypass,
                replica_groups=[list(range(NCORES))],
                ins=[wt_part.ap()], outs=[wt_h.ap()])

            # ---- main loop over token blocks ----
            for blk in range(NBLK):
                t0 = blk * T_B
                # x^T for this block: [128 d, KT, T_B] f16 via XBAR transpose
                xt = xt_pool.tile([P, KT, T_B], f16, tag="xt",
                                  name=f"xt_{blk}")
                for k in range(KT):
                    nc.sync.dma_start_transpose(
                        xt[:, k, :],
                        x_d.ap()[t0:t0 + T_B, k * P:(k + 1) * P])

                ycur = [None] * NTT
                scur = [None] * NTT
                qcur = [None] * NTT
                for op in range(NOP):
                    o0 = op * OPW
                    banks = []
                    for tt in range(NTT):
                        for os_ in range(NOS):
                            bank_t = pp.tile([P, 512], f32, tag="bank",
                                             name=f"bank_{blk}_{op}_{tt}_{os_}")
                            banks.append(bank_t)
                    # stream W^T slabs and accumulate
                    for k in range(KT):
                        wslab = wst.tile([P, OPW], f16, tag="ws",
                                         name=f"ws_{blk}_{op}_{k}")
                        nc.sync.dma_start(
                            wslab[:],
                            wt_h.ap()[k * P:(k + 1) * P, o0:o0 + OPW])
                        for tt in range(NTT):
                            for os_ in range(NOS):
                                nc.tensor.matmul(
                                    banks[tt * NOS + os_][:],
                                    xt[:, k, tt * P:(tt + 1) * P],
                                    wslab[:, os_ * 512:(os_ + 1) * 512],
                                    start=(k == 0), stop=(k == KT - 1))
                    # evacuate + stats
                    for tt in range(NTT):
                        if op == 0:
                            ycur[tt] = ypool.tile([P, DOUT], f32, tag="y",
                                                  name=f"y_{blk}_{tt}")
                            scur[tt] = stat.tile([P, NOP * NOS], f32,
                                                 tag="sums",
                                                 name=f"sums_{blk}_{tt}")
                            qcur[tt] = stat.tile([P, NOP * NOS], f32,
                                                 tag="sumsq",
                                                 name=f"sumsq_{blk}_{tt}")
                        y = ycur[tt]; sums = scur[tt]; sumsq = qcur[tt]
                        for os_ in range(NOS):
                            col = op * NOS + os_
                            zsl = y[:, o0 + os_ * 512: o0 + (os_ + 1) * 512]
                            bankap = banks[tt * NOS + os_][:]
                            nc.vector.tensor_scalar(
                                zsl, bankap, 1.0, 0.0, Alu.mult, Alu.add,
                                accum_out=sums[:, col:col + 1])
                            sq = sq_pool.tile([P, 512], f32, tag="sq",
                                              name=f"sq_{blk}_{op}_{tt}_{os_}")
                            nc.scalar.activation(
                                sq[:], zsl, Act.Square, bias=zero_t[:, 0:1],
                                accum_out=sumsq[:, col:col + 1])

                # ---- per-t-tile epilogue ----
                for tt in range(NTT):
                    y = ycur[tt]; sums = scur[tt]; sumsq = qcur[tt]
                    mu = stat.tile([P, 1], f32, tag="mu", name=f"mu_{blk}_{tt}")
                    nc.vector.tensor_reduce(
                        out=mu[:], in_=sums[:], op=Alu.add,
                        axis=mybir.AxisListType.X)
                    nc.vector.tensor_scalar(mu[:], mu[:], 1.0 / DOUT, None, Alu.mult)
                    e2 = stat.tile([P, 1], f32, tag="e2", name=f"e2_{blk}_{tt}")
                    nc.vector.tensor_reduce(
                        out=e2[:], in_=sumsq[:], op=Alu.add,
                        axis=mybir.AxisListType.X)
                    musq = stat.tile([P, 1], f32, tag="musq", name=f"musq_{blk}_{tt}")
                    nc.vector.tensor_tensor(musq[:], mu[:], mu[:], Alu.mult)
                    var = stat.tile([P, 1], f32, tag="var", name=f"var_{blk}_{tt}")
                    nc.vector.tensor_scalar(
                        var[:], e2[:], 1.0 / DOUT, None, Alu.mult)
                    nc.vector.tensor_tensor(var[:], var[:], musq[:], Alu.subtract)
                    sd = stat.tile([P, 1], f32, tag="sd", name=f"sd_{blk}_{tt}")
                    nc.scalar.activation(sd[:], var[:], Act.Sqrt, bias=eps_t[:, 0:1])
                    inv = stat.tile([P, 1], f32, tag="inv", name=f"inv_{blk}_{tt}")
                    nc.vector.reciprocal(inv[:], sd[:])
                    # normalize in place: (z - mu) * inv
                    nc.vector.tensor_scalar(
                        y[:], y[:], mu[:, 0:1], inv[:, 0:1],
                        Alu.subtract, Alu.mult)
                    # tanh(y / qs)
                    nc.scalar.activation(y[:], y[:], Act.Tanh,
                                         bias=zero_t[:, 0:1], scale=tinv[:, 0:1])
                    # k = round(tanh*127) via magic, emitted as int8
                    nc.vector.tensor_scalar(
                        y[:], y[:], 127.0, MAGIC, Alu.mult, Alu.add)
                    oi8 = opool.tile([P, DOUT], i8, tag="oi8",
                                     name=f"oi8_{blk}_{tt}")
                    nc.vector.tensor_scalar(
                        oi8[:], y[:], MAGIC, None, Alu.subtract)
                    nc.sync.dma_start(
                        out_d.ap()[blk * T_B + tt * P: blk * T_B + (tt + 1) * P, :],
                        oi8[:])

    nc.compile()
    return nc


def _pack_ternary(weight: np.ndarray) -> np.ndarray:
    """Ternarize weight [DOUT, DIN] and 2-bit-pack W^T -> [DIN, DOUT//4] u8.

    Field k of byte (d, j) holds tern_w[k*(DOUT//4) + j, d] + 1 (0/1/2).
    Matches reference: tern = where(|w| < THRESH, 0, sign(w)).
    """
    w = np.asarray(weight, dtype=np.float32)
    enc = np.ones(w.shape, dtype=np.uint8)
    enc += w >= THRESH
    enc -= w <= -THRESH
    q = DOUT // 4
    wp2 = (enc[0 * q:1 * q]
           | (enc[1 * q:2 * q] << 2)
           | (enc[2 * q:3 * q] << 4)
           | (enc[3 * q:4 * q] << 6))            # [q, DIN]
    return np.ascontiguousarray(wp2.T)           # [DIN, q]


def _host_general(x, weight, bias, scale, ln_gamma, ln_beta, quant_scale):
    """Exact host fallback for the affine-params case."""
    x2 = np.asarray(x, dtype=np.float32).reshape(-1, DIN)
    w = np.asarray(weight, dtype=np.float32)
    tern = np.where(np.abs(w) < THRESH, 0.0, np.sign(w)).astype(np.float32)
    y = x2 @ tern.T
    y += np.asarray(bias, dtype=np.float32)
    y *= np.asarray(scale, dtype=np.float32)
    mu = y.mean(-1, keepdims=True)
    var = y.var(-1, keepdims=True)
    y = (y - mu) / np.sqrt(var + LN_EPS)
    y *= np.asarray(ln_gamma, dtype=np.float32)
    y += np.asarray(ln_beta, dtype=np.float32)
    y = np.tanh(y / np.asarray(quant_scale, dtype=np.float32))
    y = np.round(y * 127.0, out=y)
    y /= np.float32(127.0)
    return y.reshape(B, S, DOUT).astype(np.float32)


def kernel(x, weight, bias, scale, ln_gamma, ln_beta, quant_scale):
    trivial = (
        not np.any(bias) and not np.any(ln_beta)
        and np.all(scale == 1.0) and np.all(ln_gamma == 1.0)
    )
    if not trivial:
        return _host_general(x, weight, bias, scale, ln_gamma, ln_beta,
                             quant_scale)

    t_start = time.perf_counter()
    if "prog" not in _CACHE:
        _CACHE["prog"] = _build()
    nc = _CACHE["prog"]

    wp = _pack_ternary(weight)
    qs = np.asarray(quant_scale, dtype=np.float32)
    nsh = DIN // NCORES
    x3 = np.asarray(x).reshape(NCORES, T, DIN)
    out = np.empty((NCORES, T, DOUT), np.float32)
    stamps = [None] * NCHUNK

    def chunk_worker(ci):
        t0 = ci * T_C
        tw0 = time.perf_counter()
        xh = x3[:, t0:t0 + T_C, :].astype(np.float16)
        in_maps = [
            {"x": xh[c], "wp": wp[c * nsh:(c + 1) * nsh], "qs": qs}
            for c in range(NCORES)
        ]
        tw1 = time.perf_counter()
        res = run_bass_kernel_spmd(nc, in_maps, list(range(NCORES)))
        tw2 = time.perf_counter()
        for c in range(NCORES):
            np.divide(res.results[c]["out"], np.float32(127.0),
                      out=out[c, t0:t0 + T_C])
        stamps[ci] = (tw0 - t_start, tw1 - tw0, tw2 - tw1,
                      time.perf_counter() - tw2)

    if NCHUNK == 1:
        chunk_worker(0)
    else:
        with ThreadPoolExecutor(NCHUNK) as ex:
            list(ex.map(chunk_worker, range(NCHUNK)))

    print("kernel chunks (start/xf16/run/outcvt): "
          + "  ".join(f"[{ci}] " + "/".join(f"{v:.2f}" for v in st)
                      for ci, st in enumerate(stamps) if st),
          file=sys.stderr)
    return out.reshape(B, S, DOUT)
